# revision 11
# baseline (speedup 1.0000x reference)
"""Trainium2 Bass kernel for nn_InterpolatorMask (embedding_lookup).

reference:  ind = floor((x - x0)/dx)
            out = sum(roll(mask, ind) * yOrig)   (0 if x outside [x0, xMax))

The roll is absorbed into host-side sharding: core c receives the slice
rolled[c*S:(c+1)*S] where rolled[i] = mask[(i - ind) mod N].  Each core
then computes a plain dot product of its yOrig shard with its rolled-mask
shard — a pure memory-bound streaming multiply-reduce (16 MiB per core)
— and returns 128 partition-partials.  The host sums the 8*128 partials
(the "all-reduce of M scalars" step) and applies the validity predicate.

Raw Bass (no TileContext: its kernel-tail drain emits more sem waits
than this walrus build encodes).  Double-buffered sync-engine DMA with
per-slot semaphores; fused multiply+row-reduce on DVE via
scalar_tensor_tensor.

Self-contained: shapes/sharding hardcoded for N = 2^24, 8 cores.
"""

import numpy as np

N = 16_777_216          # 2^24 grid length
NCORES = 8
S = N // NCORES         # 2,097,152 elements per core
P = 128                 # SBUF partitions
F = 2048                # free-dim elements per tile  -> tile = 1 MiB
NTILES = S // (P * F)   # 8 tiles per input array per core
NBUF = 4                # DMA slots in flight per array

_BUILD_CACHE = {}


def build_bass(reps=1, f=F, nbuf=NBUF, compute=True, dual=False):
    """Build (and cache) the per-core Bass module.

    reps > 1 repeats the streaming pass over the same inputs — used only
    for slope-based device-time measurement (overhead cancels).
    f/nbuf/compute/dual parametrize the kernel for perf experiments; the
    graded path uses the defaults.  dual=True issues the m-array DMAs
    from the gpsimd engine instead of sync (parallel issue, more queues).
    """
    key = (reps, f, nbuf, compute, dual)
    if key in _BUILD_CACHE:
        return _BUILD_CACHE[key]
    ntiles = S // (P * f)

    import concourse.bass as bass
    import concourse.mybir as mybir

    f32 = mybir.dt.float32
    nc = bass.Bass()
    y = nc.declare_dram_parameter("y", [S], f32, isOutput=False)
    m = nc.declare_dram_parameter("m", [S], f32, isOutput=False)
    out = nc.declare_dram_parameter("out", [P, 1], f32, isOutput=True)

    y3 = y[:].rearrange("(n p f) -> n p f", p=P, f=f)
    m3 = m[:].rearrange("(n p f) -> n p f", p=P, f=f)

    from contextlib import ExitStack

    NT = ntiles * reps

    with ExitStack() as ctx:
        ybuf = ctx.enter_context(nc.sbuf_tensor([P, nbuf * f], f32))
        mbuf = ctx.enter_context(nc.sbuf_tensor([P, nbuf * f], f32))
        prod = ctx.enter_context(nc.sbuf_tensor([P, f], f32))
        acc = ctx.enter_context(nc.sbuf_tensor([P, ntiles], f32))
        col = ctx.enter_context(nc.sbuf_tensor([P, 1], f32))
        vec_sem = ctx.enter_context(nc.semaphore("vec_sem"))
        out_sem = ctx.enter_context(nc.semaphore("out_sem"))
        slot_sems = [
            ctx.enter_context(nc.semaphore(f"slot{b}")) for b in range(nbuf)
        ]
        with nc.Block() as block:

            @block.sync
            def _(sync):
                for i in range(NT):
                    b = i % nbuf
                    t = i % ntiles
                    if i >= nbuf:
                        # slot reuse: wait until DVE consumed tile i-NBUF
                        sync.wait_ge(vec_sem, i - nbuf + 1)
                    sync.dma_start(
                        out=ybuf[:, b * f : (b + 1) * f], in_=y3[t, :, :]
                    ).then_inc(slot_sems[b], 16)
                    if not dual:
                        sync.dma_start(
                            out=mbuf[:, b * f : (b + 1) * f], in_=m3[t, :, :]
                        ).then_inc(slot_sems[b], 16)
                sync.wait_ge(vec_sem, NT + 1)
                sync.dma_start(out=out[:, :], in_=col[:, :]).then_inc(out_sem, 16)
                sync.wait_ge(out_sem, 16)

            if dual:

                @block.gpsimd
                def _(gpsimd):
                    for i in range(NT):
                        b = i % nbuf
                        t = i % ntiles
                        if i >= nbuf:
                            gpsimd.wait_ge(vec_sem, i - nbuf + 1)
                        gpsimd.dma_start(
                            out=mbuf[:, b * f : (b + 1) * f], in_=m3[t, :, :]
                        ).then_inc(slot_sems[b], 16)

            @block.vector
            def _(vector):
                for i in range(NT):
                    b = i % nbuf
                    t = i % ntiles
                    # both DMAs of this slot's (i // NBUF + 1)-th use done
                    vector.wait_ge(slot_sems[b], 32 * (i // nbuf + 1))
                    if compute:
                        nc.vector.scalar_tensor_tensor(
                            out=prod[:, :],
                            in0=ybuf[:, b * f : (b + 1) * f],
                            scalar=1.0,
                            in1=mbuf[:, b * f : (b + 1) * f],
                            op0=mybir.AluOpType.bypass,
                            op1=mybir.AluOpType.mult,
                            accum_out=acc[:, t : t + 1],
                        ).then_inc(vec_sem, 1)
                    else:
                        vector.sem_inc(vec_sem, 1)
                # accum_out writes land only at a drain; barrier before reading acc
                nc.vector.drain()
                nc.vector.reduce_sum(
                    out=col[:], in_=acc[:, :], axis=mybir.AxisListType.X
                )
                nc.vector.drain().then_inc(vec_sem, 1)

    _BUILD_CACHE[key] = nc
    return nc


def run_spmd(in_maps, trace=False, **kw):
    from concourse.bass_utils import run_bass_kernel_spmd

    nc = build_bass()
    return run_bass_kernel_spmd(nc, in_maps, list(range(NCORES)), trace=trace, **kw)


def make_in_maps(yOrig, mask, ind):
    rolled = np.roll(np.ascontiguousarray(mask, dtype=np.float32), ind)
    ys = np.ascontiguousarray(yOrig, dtype=np.float32).reshape(NCORES, S)
    ms = rolled.reshape(NCORES, S)
    return [{"y": ys[c], "m": ms[c]} for c in range(NCORES)]


def finish(results, valid):
    if not valid:
        return np.zeros((), dtype=np.float32)
    total = np.float32(0.0)
    for r in results:
        total = np.float32(total + np.float32(r["out"].sum(dtype=np.float64)))
    return np.asarray(total, dtype=np.float32).reshape(())


def kernel(x, xOrig, yOrig, mask):
    x = np.float32(np.asarray(x))
    xOrig = np.asarray(xOrig)
    x0 = np.float32(xOrig[0])
    dx = np.float32(np.float32(xOrig[1]) - x0)
    xMax = np.float32(xOrig[-1])
    ind = int(np.floor((x - x0) / dx))
    valid = bool(x >= x0) and bool(x < xMax)

    in_maps = make_in_maps(yOrig, mask, ind)
    results = run_spmd(in_maps).results
    return finish(results, valid)
